# revision 2
# baseline (speedup 1.0000x reference)
"""Trainium2 Bass kernel v2 for the dense transformer block (pre-LN GPT).

Reference: x = x + attn(LN1(x)); x = x + mlp(LN2(x))  (causal, tanh-gelu)
B=2, T=2048, C=1024, H=16, DH=64, FFN=4096, fp32 I/O.

Distribution (8 cores, no collectives): core c = (batch b=c//4, j=c%4),
queries = 4 strided q-tiles {j, 4+j, 8+j, 12+j} of 128 tokens. Attention
level t in 0..3 processes q-tile (4t+j) against key tiles 0..4t+3 (uniform
62.5% causal coverage). Causality on the last 4 key slots of each level is
data-driven: per-core mask tiles (zero / lower-tri NEG / all NEG) added
into PSUM with an identity-stationary matmul before the scores accumulate.

Precision plan (cost model: fp8e4 DoubleRow matmul = 0.5 cyc/row):
  - LN1 computed on host (free), xhat / weights quantized fp8e4.
  - QKV + attn-proj + AV: single fp8 DR. Scores: bf16 (k-stationary).
  - FFN: both operands split hi+lo fp8, 3-pass DR (hi*hi + lo*hi + hi*lo).
  - Softmax: exp(s - 3) on Act engine, denominator rides as a 65th ones
    column through the AV matmul; normalization via reciprocal + PE
    row-broadcast per head.
"""

import math
import numpy as np
import ml_dtypes

B, T, C = 2, 2048, 1024
H, DH = 16, 64
F = 4 * C
Q = 512            # query tokens per core (4 tiles of 128)
NCORES = 8
KP = 4             # contraction pairs for C=1024 (4 x (2x128))
FKP = 16           # contraction pairs for F=4096
LN_EPS = 1e-5
NEG = -30000.0
SHIFT = 3.0
# power-of-2 pre-scales keeping fp8 operands out of the subnormal range
SQ, SK, SV, SP, SF, SO = 128.0, 16.0, 16.0, 16.0, 16.0, 32.0
SA = 2.0   # activation hi/lo streams (xhat2, g) scaled by 2
SY = 2.0   # v carries 2v; rden carries 2/den -> ytil = 4y

_cache = {}


def _build():
    import concourse.mybir as mybir
    import concourse.tile as tile
    from concourse import bacc

    f32 = mybir.dt.float32
    bf16 = mybir.dt.bfloat16
    f8 = mybir.dt.float8e4
    Alu = mybir.AluOpType
    Act = mybir.ActivationFunctionType
    DR = mybir.MatmulPerfMode.DoubleRow

    nc = bacc.Bacc("TRN2", target_bir_lowering=False, debug=False,
                   num_devices=NCORES)

    # ---------------- DRAM tensors ----------------
    xh_d = nc.dram_tensor("xh", [KP * 128, 2 * T], f8, kind="ExternalInput")
    xq_d = nc.dram_tensor("xq", [KP * 128, 2 * Q], f8, kind="ExternalInput")
    wq_d = nc.dram_tensor("wq", [KP * 128, 2 * C], f8, kind="ExternalInput")
    wk_d = nc.dram_tensor("wk", [KP * 128, 2 * C], f8, kind="ExternalInput")
    wv_d = nc.dram_tensor("wv", [KP * 128, 2 * C], f8, kind="ExternalInput")
    wp_d = nc.dram_tensor("wp", [KP * 128, 2 * C], f8, kind="ExternalInput")
    wfh_d = nc.dram_tensor("wfh", [KP * 128, 2 * F], f8, kind="ExternalInput")
    wfl_d = nc.dram_tensor("wfl", [KP * 128, 2 * F], f8, kind="ExternalInput")
    woh_d = nc.dram_tensor("woh", [FKP * 128, 2 * C], f8, kind="ExternalInput")
    wol_d = nc.dram_tensor("wol", [FKP * 128, 2 * C], f8, kind="ExternalInput")
    msk_d = nc.dram_tensor("msk", [128, 4 * 128], bf16, kind="ExternalInput")
    eye_d = nc.dram_tensor("eye", [128, 128], bf16, kind="ExternalInput")
    w2r_d = nc.dram_tensor("w2r", [1, C], f32, kind="ExternalInput")   # ln2_w row
    b2r_d = nc.dram_tensor("b2r", [1, C], f32, kind="ExternalInput")   # ln2_b row
    xT_d = nc.dram_tensor("xT", [C, Q], f32, kind="ExternalInput")
    out_d = nc.dram_tensor("outT", [C, Q], f32, kind="ExternalOutput")

    with tile.TileContext(nc) as tc:
        # ---------------- persistent SBUF ----------------
        cst = tc.alloc_tile_pool(name="cst", bufs=1, side="left")
        eye = cst.tile([128, 128], bf16, name="eye", tag="eye")
        msk = cst.tile([128, 4, 128], bf16, name="msk", tag="msk")
        ones_r64 = cst.tile([1, 64], bf16, name="o64", tag="o64")
        ones_col = cst.tile([128, 1], bf16, name="oc", tag="oc")
        w2r = cst.tile([1, C], f32, name="w2r", tag="w2r")
        b2r = cst.tile([1, C], f32, name="b2r", tag="b2r")
        nc.sync.dma_start(eye[:], eye_d[:])
        nc.sync.dma_start(msk[:], msk_d[:])
        nc.sync.dma_start(w2r[:], w2r_d[:])
        nc.sync.dma_start(b2r[:], b2r_d[:])
        nc.vector.memset(ones_r64[:], 1.0)
        nc.vector.memset(ones_col[:], 1.0)
        shift_t = cst.tile([128, 1], f32, name="shift", tag="shift")
        nc.vector.memset(shift_t[:], -SHIFT)

        # FFN weights: allocated under the attention pools so they persist
        # after those release; DMAs are emitted later (after the projection
        # input DMAs) so they don't block the critical path.
        p_wf = tc.alloc_tile_pool(name="pwf", bufs=1, side="left")
        wf8h = [p_wf.tile([128, 2, F], f8, name=f"wfh{k}", tag=f"wfh{k}")
                for k in range(KP)]
        wf8l = [p_wf.tile([128, 2, F], f8, name=f"wfl{k}", tag=f"wfl{k}")
                for k in range(KP)]

        # attention-phase inputs (freed before FFN)
        p_xh = tc.alloc_tile_pool(name="pxh", bufs=1, side="left")
        xh8 = [p_xh.tile([128, 2, T], f8, name=f"xh{k}", tag=f"xh{k}")
               for k in range(KP)]
        p_xq = tc.alloc_tile_pool(name="pxq", bufs=1, side="left")
        xq8 = [p_xq.tile([128, 2, Q], f8, name=f"xq{k}", tag=f"xq{k}")
               for k in range(KP)]
        p_wq = tc.alloc_tile_pool(name="pwq", bufs=1, side="left")
        wq8 = [p_wq.tile([128, 2, C], f8, name=f"wq{k}", tag=f"wq{k}")
               for k in range(KP)]
        p_wk = tc.alloc_tile_pool(name="pwk", bufs=1, side="left")
        wk8 = [p_wk.tile([128, 2, C], f8, name=f"wk{k}", tag=f"wk{k}")
               for k in range(KP)]
        p_wv = tc.alloc_tile_pool(name="pwv", bufs=1, side="left")
        wv8 = [p_wv.tile([128, 2, C], f8, name=f"wv{k}", tag=f"wv{k}")
               for k in range(KP)]

        for k in range(KP):
            r = slice(k * 128, (k + 1) * 128)
            nc.sync.dma_start(xq8[k][:], xq_d[r, :])
            nc.sync.dma_start(wq8[k][:], wq_d[r, :])
        for k in range(KP):
            r = slice(k * 128, (k + 1) * 128)
            nc.sync.dma_start(xh8[k][:], xh_d[r, :])
            nc.sync.dma_start(wk8[k][:], wk_d[r, :])
        for k in range(KP):
            r = slice(k * 128, (k + 1) * 128)
            nc.sync.dma_start(wv8[k][:], wv_d[r, :])

        # persistent activation tiles
        p_qt = tc.alloc_tile_pool(name="pqt", bufs=1, side="left")
        qT = [p_qt.tile([128, Q], bf16, name=f"qT{m}", tag=f"qT{m}")
              for m in range(8)]
        p_kt = tc.alloc_tile_pool(name="pkt", bufs=1, side="left")
        kT = [p_kt.tile([128, 4, 512], bf16, name=f"kT{m}", tag=f"kT{m}")
              for m in range(8)]
        p_v = tc.alloc_tile_pool(name="pv8", bufs=1, side="left")
        v8 = [p_v.tile([128, 2, H, DH + 1], f8, name=f"v8{p}", tag=f"v8{p}")
              for p in range(8)]
        p_yt = tc.alloc_tile_pool(name="pyt", bufs=1, side="right")
        yt8 = [p_yt.tile([128, 2, Q], f8, name=f"yt{p}", tag=f"yt{p}")
               for p in range(KP)]
        p_wp = tc.alloc_tile_pool(name="pwp", bufs=1, side="right")
        wp8 = [p_wp.tile([128, 2, C], f8, name=f"wp{k}", tag=f"wp{k}")
               for k in range(KP)]
        for k in range(KP):
            nc.sync.dma_start(wp8[k][:], wp_d[k * 128:(k + 1) * 128, :])
        # FFN weight prefetch: emitted after all projection inputs so the
        # (serialized) DMA engines load those first; overlaps attention.
        for k in range(KP):
            r = slice(k * 128, (k + 1) * 128)
            nc.sync.dma_start(wf8h[k][:], wfh_d[r, :])
        for k in range(KP):
            r = slice(k * 128, (k + 1) * 128)
            nc.sync.dma_start(wf8l[k][:], wfl_d[r, :])

        # ---------------- phase 1: Q and V projections ----------------
        with tc.tile_pool(name="ppq", bufs=2, space="PSUM") as ppq:
            for ft in range(8):
                fs = slice(ft * 128, (ft + 1) * 128)
                ps = ppq.tile([128, Q], f32, name="pq", tag="pq")
                for k in range(KP):
                    nc.tensor.matmul(ps[:], wq8[k][:, :, fs], xq8[k][:],
                                     start=(k == 0), stop=(k == KP - 1),
                                     perf_mode=DR)
                nc.vector.tensor_scalar_mul(qT[ft][:], ps[:], 1.0 / SQ)

        # V: out[keys, vfeat] via xhat-stationary; vfeat = 64h+d, +65th ones col
        with tc.tile_pool(name="ppv", bufs=2, space="PSUM") as ppv:
            for kt in range(16):
                ks = slice(kt * 128, (kt + 1) * 128)
                ps = ppv.tile([128, 16, 64], f32, name="pvv", tag="pvv")
                for vc in range(2):
                    for k in range(KP):
                        nc.tensor.matmul(ps[:, vc * 8:(vc + 1) * 8, :],
                                         xh8[k][:, :, ks],
                                         wv8[k][:, :, vc * 512:(vc + 1) * 512],
                                         start=(k == 0), stop=(k == KP - 1),
                                         perf_mode=DR)
                # [128, 16, 64] -> v8[kt//2][:, kt%2, :, 0:64] (stride 65)
                nc.vector.tensor_scalar_mul(
                    v8[kt // 2][:, kt % 2, :, 0:DH], ps[:], SY / SV)
            for p in range(8):
                nc.vector.memset(v8[p][:, :, :, DH:DH + 1], 1.0)

        # ---------------- phase 2: attention (K proj interleaved) ----------
        # per head h: level t: scores into psum group(s), exp -> a8, AV-DR.
        # K projection for feature tile ft is emitted just before the scores
        # of its two heads, feeding the PE while Act drains exp work.
        nheads = H

        with tc.tile_pool(name="psc", bufs=2, space="PSUM") as psc, \
             tc.tile_pool(name="psy", bufs=2, space="PSUM") as psy, \
             tc.tile_pool(name="pa8", bufs=3, side="right") as pa8, \
             tc.tile_pool(name="pnrm", bufs=2, side="right") as pnrm, \
             tc.tile_pool(name="ppk", bufs=1, space="PSUM") as ppk:

            def kproj(ft):
                fs = slice(ft * 128, (ft + 1) * 128)
                for half in range(2):
                    ps = ppk.tile([128, 2, 512], f32, name="pk", tag="pk")
                    for kc2 in range(2):
                        kc = half * 2 + kc2
                        cs = slice(kc * 512, (kc + 1) * 512)
                        for k in range(KP):
                            nc.tensor.matmul(ps[:, kc2, :], wk8[k][:, :, fs],
                                             xh8[k][:, :, cs],
                                             start=(k == 0),
                                             stop=(k == KP - 1),
                                             perf_mode=DR)
                    nc.vector.tensor_scalar_mul(
                        kT[ft][:, half * 2:half * 2 + 2, :], ps[:], 1.0 / SK)

            def scores_head(h, y_ps):
                ft, po = h // 2, (h % 2) * 64
                groups = []
                for t in range(4):
                    nslot = 4 * (t + 1)
                    qs = slice(t * 128, (t + 1) * 128)
                    for g0 in range(0, nslot, 8):
                        gn = min(8, nslot - g0)
                        s_ps = psc.tile([128, 8, 128], f32, name="s", tag="s")
                        a8 = pa8.tile([128, 8, 128], f8, name="a8", tag="a8")
                        for si in range(gn):
                            s = g0 + si
                            masked = s >= nslot - 4
                            if masked:
                                nc.tensor.matmul(
                                    s_ps[:, si, :], eye[:],
                                    msk[:, s - (nslot - 4), :],
                                    start=True, stop=False)
                            nc.tensor.matmul(
                                s_ps[:, si, :],
                                kT[ft][po:po + 64, s // 4,
                                       (s % 4) * 128:(s % 4) * 128 + 128],
                                qT[ft][po:po + 64, qs],
                                start=not masked, stop=True)
                        nc.scalar.activation(a8[:, 0:gn, :], s_ps[:, 0:gn, :],
                                             Act.Exp, bias=shift_t[:])
                        groups.append((t, g0, gn, a8))
                return groups

            def av_head(h, y_ps, groups):
                for (t, g0, gn, a8) in groups:
                    qs = slice(t * 128, (t + 1) * 128)
                    npair_t = 2 * (t + 1)
                    for pi in range(gn // 2):
                        p = (g0 // 2) + pi
                        nc.tensor.matmul(
                            y_ps[0:65, qs], v8[p][:, :, h, :],
                            a8[:, 2 * pi:2 * pi + 2, :],
                            start=(p == 0), stop=(p == npair_t - 1),
                            perf_mode=DR)

            def norm_head(h, y_ps):
                rd = pnrm.tile([1, Q], f32, name="rd", tag="rd")
                rdb = pnrm.tile([1, Q], bf16, name="rdb", tag="rdb")
                rb = pnrm.tile([64, Q], bf16, name="rb", tag="rb")
                nc.vector.reciprocal(rd[:], y_ps[64:65, :])
                nc.vector.tensor_scalar_mul(rdb[:], rd[:], SY)
                nc.tensor.matmul(y_ps[64:128, :], ones_r64[:], rdb[:],
                                 start=True, stop=True)
                nc.vector.tensor_copy(rb[:], y_ps[64:128, :])
                pp, ss, rr = h // 4, (h // 2) % 2, (h % 2) * 64
                nc.vector.tensor_tensor(
                    yt8[pp][rr:rr + 64, ss, :], y_ps[0:64, :], rb[:],
                    Alu.mult)

            prev = None
            for h in range(nheads):
                if h % 2 == 0:
                    kproj(h // 2)
                y_ps = psy.tile([128, Q], f32, name="y", tag="y")
                groups = scores_head(h, y_ps)
                if prev is not None:
                    av_head(prev[0], prev[1], prev[2])
                    norm_head(prev[0], prev[1])
                prev = (h, y_ps, groups)
            av_head(prev[0], prev[1], prev[2])
            norm_head(prev[0], prev[1])

        p_v.release()
        p_kt.release()
        p_qt.release()
        p_wv.release()
        p_wk.release()
        p_wq.release()
        p_xq.release()
        p_xh.release()

        # ---------------- phase 3: attn proj + residual + LN2 ----------------
        p_x2 = tc.alloc_tile_pool(name="px2", bufs=1, side="right")
        xT_sb = [p_x2.tile([128, Q], f32, name=f"xT{m}", tag=f"xT{m}")
                 for m in range(8)]
        for m in range(8):
            nc.sync.dma_start(xT_sb[m][:], xT_d[m * 128:(m + 1) * 128, :])
        x2_sb = [p_x2.tile([128, Q], f32, name=f"x2{m}", tag=f"x2{m}")
                 for m in range(8)]
        x2b = [p_x2.tile([128, Q], bf16, name=f"x2b{m}", tag=f"x2b{m}")
               for m in range(8)]
        xq2h = [p_x2.tile([128, 2, Q], f8, name=f"q2h{k}", tag=f"q2h{k}")
                for k in range(KP)]
        xq2l = [p_x2.tile([128, 2, Q], f8, name=f"q2l{k}", tag=f"q2l{k}")
                for k in range(KP)]
        mu2 = p_x2.tile([1, Q], f32, name="mu2", tag="mu2")
        e22 = p_x2.tile([1, Q], f32, name="e22", tag="e22")
        rr2 = p_x2.tile([1, Q], f32, name="rr2", tag="rr2")
        mr2 = p_x2.tile([1, Q], f32, name="mr2", tag="mr2")
        mr2b = p_x2.tile([1, Q], bf16, name="mr2b", tag="mr2b")
        rr2b = p_x2.tile([1, Q], bf16, name="rr2b", tag="rr2b")
        S_sb = p_x2.tile([128, Q], bf16, name="S", tag="S")      # w2 x r2
        M_sb = p_x2.tile([128, Q], bf16, name="M", tag="M")      # w2 x mu2 r2 - b2
        t2_sb = p_x2.tile([128, Q], bf16, name="t2", tag="t2")

        with tc.tile_pool(name="pp3", bufs=2, space="PSUM") as pp3, \
             tc.tile_pool(name="pst", bufs=1, space="PSUM") as pst, \
             tc.tile_pool(name="psq", bufs=2, side="right") as psq:
            s2_ps = pst.tile([1, Q], f32, name="s2", tag="s2")
            q2_ps = pst.tile([1, Q], f32, name="q2", tag="q2")
            for ft in range(8):
                fs = slice(ft * 128, (ft + 1) * 128)
                ps = pp3.tile([128, Q], f32, name="pj", tag="pj")
                for k in range(KP):
                    nc.tensor.matmul(ps[:], wp8[k][:, :, fs], yt8[k][:],
                                     start=(k == 0), stop=(k == KP - 1),
                                     perf_mode=DR)
                nc.vector.scalar_tensor_tensor(
                    x2_sb[ft][:], ps[:], 1.0 / (SY * SY * SP), xT_sb[ft][:],
                    Alu.mult, Alu.add)
                nc.vector.tensor_copy(x2b[ft][:], x2_sb[ft][:])
                sq = psq.tile([128, Q], bf16, name="sq", tag="sq")
                nc.scalar.square(sq[:], x2b[ft][:])
                nc.tensor.matmul(s2_ps[:], ones_col[:], x2b[ft][:],
                                 start=(ft == 0), stop=(ft == 7))
                nc.tensor.matmul(q2_ps[:], ones_col[:], sq[:],
                                 start=(ft == 0), stop=(ft == 7))
            nc.vector.tensor_scalar_mul(mu2[:], s2_ps[:], 1.0 / C)
            nc.vector.tensor_scalar_mul(e22[:], q2_ps[:], 1.0 / C)

        # rr2 = 1/sqrt(var+eps); mr2 = mu2*rr2
        eps_t = p_x2.tile([1, 1], f32, name="eps", tag="eps")
        nc.vector.memset(eps_t[:], LN_EPS)
        nc.vector.tensor_tensor(rr2[:], mu2[:], mu2[:], Alu.mult)
        nc.vector.tensor_tensor(rr2[:], e22[:], rr2[:], Alu.subtract)
        nc.scalar.activation(rr2[:], rr2[:], Act.Sqrt, bias=eps_t[:])
        nc.vector.reciprocal(rr2[:], rr2[:])
        nc.vector.tensor_tensor(mr2[:], mu2[:], rr2[:], Alu.mult)
        nc.vector.tensor_copy(rr2b[:], rr2[:])
        nc.vector.tensor_copy(mr2b[:], mr2[:])

        # S = w2 (x) rr2 ; M = w2 (x) mr2 - b2 (x) 1   (PE broadcasts)
        w2b = p_x2.tile([1, C], bf16, name="w2b", tag="w2b")
        b2b = p_x2.tile([1, C], bf16, name="b2b", tag="b2b")
        onerow = p_x2.tile([1, Q], bf16, name="onerow", tag="onerow")
        nc.vector.tensor_copy(w2b[:], w2r[:])
        nc.vector.tensor_scalar_mul(b2b[:], b2r[:], -1.0)
        nc.vector.memset(onerow[:], 1.0)
        with tc.tile_pool(name="pbc2", bufs=2, space="PSUM") as pbc2:
            for ft in range(8):
                fs = slice(ft * 128, (ft + 1) * 128)
                sp = pbc2.tile([128, Q], f32, name="sp", tag="sp")
                mp = pbc2.tile([128, Q], f32, name="mp", tag="mp")
                nc.tensor.matmul(sp[:], w2b[:, fs], rr2b[:],
                                 start=True, stop=True)
                nc.tensor.matmul(mp[:], w2b[:, fs], mr2b[:],
                                 start=True, stop=False)
                nc.tensor.matmul(mp[:], b2b[:, fs], onerow[:],
                                 start=False, stop=True,
                                 skip_group_check=True)
                nc.vector.tensor_copy(S_sb[:], sp[:])
                nc.vector.tensor_copy(M_sb[:], mp[:])
                # xhat2 = x2*S - M, then hi/lo fp8 split
                kp, sl = ft // 2, ft % 2
                nc.vector.tensor_tensor(t2_sb[:], x2_sb[ft][:], S_sb[:],
                                        Alu.mult)
                nc.vector.tensor_tensor(t2_sb[:], t2_sb[:], M_sb[:],
                                        Alu.subtract)
                nc.vector.tensor_scalar_mul(xq2h[kp][:, sl, :], t2_sb[:], SA)
                nc.vector.scalar_tensor_tensor(
                    xq2l[kp][:, sl, :], t2_sb[:], SA,
                    xq2h[kp][:, sl, :], Alu.mult, Alu.subtract)

        # ---------------- phase 4: FFN ----------------
        p_g = tc.alloc_tile_pool(name="pg", bufs=1, side="right")
        g8h = [p_g.tile([128, 2, Q], f8, name=f"g8h{k}", tag=f"g8h{k}")
               for k in range(FKP)]
        g8l = [p_g.tile([128, 2, Q], f8, name=f"g8l{k}", tag=f"g8l{k}")
               for k in range(FKP)]
        with tc.tile_pool(name="pgb", bufs=4, side="right") as pgb, \
             tc.tile_pool(name="pph", bufs=3, space="PSUM") as pph:
            gb = None
            for ht in range(32):
                hs = slice(ht * 128, (ht + 1) * 128)
                ps = pph.tile([128, Q], f32, name="ph", tag="ph")
                for k in range(KP):
                    nc.tensor.matmul(ps[:], wf8h[k][:, :, hs], xq2h[k][:],
                                     start=(k == 0), stop=False,
                                     perf_mode=DR)
                for k in range(KP):
                    nc.tensor.matmul(ps[:], wf8h[k][:, :, hs], xq2l[k][:],
                                     start=False, stop=False, perf_mode=DR)
                for k in range(KP):
                    nc.tensor.matmul(ps[:], wf8l[k][:, :, hs], xq2h[k][:],
                                     start=False, stop=(k == KP - 1),
                                     perf_mode=DR)
                if ht % 2 == 0:
                    gb = pgb.tile([128, 2, Q], bf16, name="gb", tag="gb")
                nc.scalar.activation(gb[:, ht % 2, :], ps[:],
                                     Act.Gelu_apprx_tanh,
                                     scale=1.0 / (SA * SF))
                if ht % 2 == 1:
                    kp = ht // 2
                    nc.vector.tensor_scalar_mul(g8h[kp][:], gb[:], SA)
                    nc.vector.scalar_tensor_tensor(
                        g8l[kp][:], gb[:], SA, g8h[kp][:],
                        Alu.mult, Alu.subtract)
        p_wf.release()
        p_wo = tc.alloc_tile_pool(name="pwo", bufs=8, side="left")

        with tc.tile_pool(name="ppo", bufs=1, space="PSUM") as ppo, \
             tc.tile_pool(name="pout", bufs=4, side="right") as pout:
            o_ps = [ppo.tile([128, Q], f32, name=f"o{m}", tag=f"o{m}")
                    for m in range(8)]
            for k in range(FKP):
                r = slice(k * 128, (k + 1) * 128)
                woh_t = p_wo.tile([128, 2, C], f8, name="woh", tag="woh")
                wol_t = p_wo.tile([128, 2, C], f8, name="wol", tag="wol")
                nc.sync.dma_start(woh_t[:], woh_d[r, :])
                nc.sync.dma_start(wol_t[:], wol_d[r, :])
                for ft in range(8):
                    fs = slice(ft * 128, (ft + 1) * 128)
                    nc.tensor.matmul(o_ps[ft][:], woh_t[:, :, fs], g8h[k][:],
                                     start=(k == 0), stop=False,
                                     perf_mode=DR)
                    nc.tensor.matmul(o_ps[ft][:], woh_t[:, :, fs], g8l[k][:],
                                     start=False, stop=False, perf_mode=DR)
                    nc.tensor.matmul(o_ps[ft][:], wol_t[:, :, fs], g8h[k][:],
                                     start=False, stop=(k == FKP - 1),
                                     perf_mode=DR)
            for ft in range(8):
                ot = pout.tile([128, Q], f32, name="ot", tag="ot")
                nc.vector.scalar_tensor_tensor(
                    ot[:], o_ps[ft][:], 1.0 / (SA * SO), x2_sb[ft][:],
                    Alu.mult, Alu.add)
                nc.sync.dma_start(out_d[ft * 128:(ft + 1) * 128, :], ot[:])

        p_g.release()
        p_x2.release()
        p_wp.release()
        p_yt.release()
        p_wo.release()
        cst.release()

    nc.compile()
    return nc


def _prep_inputs(x, w_attn, w_proj, w_fc, w_fc_proj,
                 ln1_w, ln1_b, ln2_w, ln2_b):
    f8 = ml_dtypes.float8_e4m3
    scale = 1.0 / math.sqrt(DH)

    def pack_w(W, npair):
        # [K, N] -> [npair*128, 2*N] pair-interleaved rows
        K, N = W.shape
        return np.ascontiguousarray(
            W.reshape(npair, 2, 128, N).transpose(0, 2, 1, 3)
             .reshape(npair * 128, 2 * N))

    def hi_lo(W):
        hi = W.astype(f8)
        lo = (W - hi.astype(np.float32)).astype(f8)
        return hi, lo

    wq = pack_w(w_attn[:, :C] * (scale * SQ), KP).astype(f8)
    wk = pack_w(w_attn[:, C:2 * C] * SK, KP).astype(f8)
    wv = pack_w(w_attn[:, 2 * C:] * SV, KP).astype(f8)
    wp = pack_w(w_proj * SP, KP).astype(f8)
    wfh, wfl = hi_lo(pack_w(w_fc * SF, KP))
    woh, wol = hi_lo(pack_w(w_fc_proj * SO, FKP))

    eye = np.eye(128, dtype=ml_dtypes.bfloat16)
    tri = np.where(np.arange(128)[:, None] > np.arange(128)[None, :],
                   np.float32(NEG), np.float32(0.0))  # [k, q]: NEG if q < k

    # host LN1
    mu = x.mean(axis=2, keepdims=True)
    var = ((x - mu) ** 2).mean(axis=2, keepdims=True)
    xhat = ((x - mu) / np.sqrt(var + LN_EPS)) * ln1_w + ln1_b  # [B, T, C]

    in_maps = []
    for c in range(NCORES):
        b, j = c // 4, c % 4
        qsel = np.concatenate([np.arange(128 * (4 * t + j),
                                         128 * (4 * t + j) + 128)
                               for t in range(4)])
        xh = pack_w(np.ascontiguousarray(xhat[b].T), KP).astype(f8)
        xq = pack_w(np.ascontiguousarray(xhat[b][qsel].T), KP).astype(f8)
        xT = np.ascontiguousarray(x[b][qsel].T.astype(np.float32))
        m = np.zeros((128, 4, 128), dtype=np.float32)
        for i in range(4):
            if i == j:
                m[:, i, :] = tri
            elif i > j:
                m[:, i, :] = NEG
        in_maps.append({
            "xh": xh, "xq": xq, "wq": wq, "wk": wk, "wv": wv, "wp": wp,
            "wfh": wfh, "wfl": wfl, "woh": woh, "wol": wol,
            "msk": m.reshape(128, 512).astype(ml_dtypes.bfloat16),
            "eye": eye,
            "w2r": ln2_w.reshape(1, C).astype(np.float32),
            "b2r": ln2_b.reshape(1, C).astype(np.float32),
            "xT": xT,
        })
    return in_maps


def _get_nc():
    if "nc" not in _cache:
        _cache["nc"] = _build()
    return _cache["nc"]


def _get_runner():
    if "runner" in _cache:
        return _cache["runner"]
    import jax
    import numpy as _np
    from jax.sharding import Mesh, PartitionSpec
    try:
        from jax.experimental.shard_map import shard_map
    except ImportError:
        from jax.shard_map import shard_map
    import concourse.mybir as mybir
    from concourse import bass2jax

    nc = _get_nc()
    bass2jax.install_neuronx_cc_hook()

    partition_name = (nc.partition_id_tensor.name
                      if nc.partition_id_tensor else None)
    in_names, out_names, out_avals, zero_outs = [], [], [], []
    for alloc in nc.m.functions[0].allocations:
        if not isinstance(alloc, mybir.MemoryLocationSet):
            continue
        name = alloc.memorylocations[0].name
        if alloc.kind == "ExternalInput":
            if name != partition_name:
                in_names.append(name)
        elif alloc.kind == "ExternalOutput":
            shape = tuple(alloc.tensor_shape)
            dtype = mybir.dt.np(alloc.dtype)
            out_names.append(name)
            out_avals.append(jax.core.ShapedArray(shape, dtype))
            zero_outs.append(_np.zeros(shape, dtype))
    n_params = len(in_names)
    n_outs = len(out_avals)
    all_in_names = list(in_names) + list(out_names)
    if partition_name is not None:
        all_in_names.append(partition_name)
    donate = tuple(range(n_params, n_params + n_outs))

    def _body(*args):
        operands = list(args)
        if partition_name is not None:
            operands.append(bass2jax.partition_id_tensor())
        outs = bass2jax._bass_exec_p.bind(
            *operands,
            out_avals=tuple(out_avals),
            in_names=tuple(all_in_names),
            out_names=tuple(out_names),
            lowering_input_output_aliases=(),
            sim_require_finite=True,
            sim_require_nnan=True,
            nc=nc,
        )
        return tuple(outs)

    devices = jax.devices()[:NCORES]
    mesh = Mesh(_np.asarray(devices), ("core",))
    in_specs = (PartitionSpec("core"),) * (n_params + n_outs)
    out_specs = (PartitionSpec("core"),) * n_outs
    sharded = jax.jit(
        shard_map(_body, mesh=mesh, in_specs=in_specs, out_specs=out_specs,
                  check_rep=False),
        donate_argnums=donate, keep_unused=True)

    def run(in_maps):
        concat_in = [
            _np.concatenate([_np.asarray(in_maps[c][n])
                             for c in range(NCORES)], axis=0)
            for n in in_names
        ]
        concat_zeros = [
            _np.zeros((NCORES * z.shape[0], *z.shape[1:]), z.dtype)
            for z in zero_outs
        ]
        out_arrs = sharded(*concat_in, *concat_zeros)
        return [
            {n: _np.asarray(out_arrs[i]).reshape(
                NCORES, *out_avals[i].shape)[c]
             for i, n in enumerate(out_names)}
            for c in range(NCORES)
        ]

    _cache["runner"] = run
    return run


def kernel(x, w_attn, w_proj, w_fc, w_fc_proj, ln1_w, ln1_b, ln2_w, ln2_b):
    x = np.asarray(x, dtype=np.float32)
    in_maps = _prep_inputs(
        x, np.asarray(w_attn, np.float32), np.asarray(w_proj, np.float32),
        np.asarray(w_fc, np.float32), np.asarray(w_fc_proj, np.float32),
        np.asarray(ln1_w, np.float32), np.asarray(ln1_b, np.float32),
        np.asarray(ln2_w, np.float32), np.asarray(ln2_b, np.float32))
    results = _get_runner()(in_maps)
    out = np.empty((B, T, C), dtype=np.float32)
    for c in range(NCORES):
        b, j = c // 4, c % 4
        o = results[c]["outT"]  # [C, Q]
        for t in range(4):
            qt = 4 * t + j
            out[b, 128 * qt:128 * qt + 128, :] = o[:, 128 * t:128 * t + 128].T
    return out


# revision 3
# speedup vs baseline: 1.0124x; 1.0124x over previous
"""Trainium2 Bass kernel v2 for the dense transformer block (pre-LN GPT).

Reference: x = x + attn(LN1(x)); x = x + mlp(LN2(x))  (causal, tanh-gelu)
B=2, T=2048, C=1024, H=16, DH=64, FFN=4096, fp32 I/O.

Distribution (8 cores, no collectives): core c = (batch b=c//4, j=c%4),
queries = 4 strided q-tiles {j, 4+j, 8+j, 12+j} of 128 tokens. Attention
level t in 0..3 processes q-tile (4t+j) against key tiles 0..4t+3 (uniform
62.5% causal coverage). Causality on the last 4 key slots of each level is
data-driven: per-core mask tiles (zero / lower-tri NEG / all NEG) added
into PSUM with an identity-stationary matmul before the scores accumulate.

Precision plan (cost model: fp8e4 DoubleRow matmul = 0.5 cyc/row):
  - LN1 computed on host (free), xhat / weights quantized fp8e4.
  - QKV + attn-proj + AV: single fp8 DR. Scores: bf16 (k-stationary).
  - FFN: both operands split hi+lo fp8, 3-pass DR (hi*hi + lo*hi + hi*lo).
  - Softmax: exp(s - 3) on Act engine, denominator rides as a 65th ones
    column through the AV matmul; normalization via reciprocal + PE
    row-broadcast per head.
"""

import math
import numpy as np
import ml_dtypes

B, T, C = 2, 2048, 1024
H, DH = 16, 64
F = 4 * C
Q = 512            # query tokens per core (4 tiles of 128)
NCORES = 8
KP = 4             # contraction pairs for C=1024 (4 x (2x128))
FKP = 16           # contraction pairs for F=4096
LN_EPS = 1e-5
NEG = -30000.0
SHIFT = 3.0
# power-of-2 pre-scales keeping fp8 operands out of the subnormal range
SQ, SK, SV, SP, SF, SO = 128.0, 16.0, 16.0, 16.0, 16.0, 32.0
SA = 2.0   # activation hi/lo streams (xhat2, g) scaled by 2
SY = 2.0   # v carries 2v; rden carries 2/den -> ytil = 4y

_cache = {}


def _build():
    import concourse.mybir as mybir
    import concourse.tile as tile
    from concourse import bacc

    f32 = mybir.dt.float32
    bf16 = mybir.dt.bfloat16
    f8 = mybir.dt.float8e4
    Alu = mybir.AluOpType
    Act = mybir.ActivationFunctionType
    DR = mybir.MatmulPerfMode.DoubleRow

    nc = bacc.Bacc("TRN2", target_bir_lowering=False, debug=False,
                   num_devices=NCORES)

    # ---------------- DRAM tensors ----------------
    xh_d = nc.dram_tensor("xh", [KP * 128, 2 * T], f8, kind="ExternalInput")
    xq_d = nc.dram_tensor("xq", [KP * 128, 2 * Q], f8, kind="ExternalInput")
    wq_d = nc.dram_tensor("wq", [KP * 128, 2 * C], f8, kind="ExternalInput")
    wk_d = nc.dram_tensor("wk", [KP * 128, 2 * C], f8, kind="ExternalInput")
    wv_d = nc.dram_tensor("wv", [KP * 128, 2 * C], f8, kind="ExternalInput")
    wp_d = nc.dram_tensor("wp", [KP * 128, 2 * C], f8, kind="ExternalInput")
    wfh_d = nc.dram_tensor("wfh", [KP * 128, 2 * F], f8, kind="ExternalInput")
    wfl_d = nc.dram_tensor("wfl", [KP * 128, 2 * F], f8, kind="ExternalInput")
    woh_d = nc.dram_tensor("woh", [FKP * 128, 2 * C], f8, kind="ExternalInput")
    wol_d = nc.dram_tensor("wol", [FKP * 128, 2 * C], f8, kind="ExternalInput")
    msk_d = nc.dram_tensor("msk", [128, 4 * 128], bf16, kind="ExternalInput")
    eye_d = nc.dram_tensor("eye", [128, 128], bf16, kind="ExternalInput")
    w2r_d = nc.dram_tensor("w2r", [1, C], f32, kind="ExternalInput")   # ln2_w row
    b2r_d = nc.dram_tensor("b2r", [1, C], f32, kind="ExternalInput")   # ln2_b row
    xT_d = nc.dram_tensor("xT", [C, Q], f32, kind="ExternalInput")
    out_d = nc.dram_tensor("outT", [C, Q], f32, kind="ExternalOutput")

    with tile.TileContext(nc) as tc:
        # ---------------- persistent SBUF ----------------
        cst = tc.alloc_tile_pool(name="cst", bufs=1, side="left")
        eye = cst.tile([128, 128], bf16, name="eye", tag="eye")
        msk = cst.tile([128, 4, 128], bf16, name="msk", tag="msk")
        ones_r64 = cst.tile([1, 64], bf16, name="o64", tag="o64")
        ones_col = cst.tile([128, 1], bf16, name="oc", tag="oc")
        w2r = cst.tile([1, C], f32, name="w2r", tag="w2r")
        b2r = cst.tile([1, C], f32, name="b2r", tag="b2r")
        nc.sync.dma_start(eye[:], eye_d[:])
        nc.sync.dma_start(msk[:], msk_d[:])
        nc.sync.dma_start(w2r[:], w2r_d[:])
        nc.sync.dma_start(b2r[:], b2r_d[:])
        nc.vector.memset(ones_r64[:], SY)   # folds the v-scale into the bcast
        nc.vector.memset(ones_col[:], 1.0)
        shift_t = cst.tile([128, 1], f32, name="shift", tag="shift")
        nc.vector.memset(shift_t[:], -SHIFT)

        # FFN weights: allocated under the attention pools so they persist
        # after those release; DMAs are emitted later (after the projection
        # input DMAs) so they don't block the critical path.
        p_wf = tc.alloc_tile_pool(name="pwf", bufs=1, side="left")
        wf8h = [p_wf.tile([128, 2, F], f8, name=f"wfh{k}", tag=f"wfh{k}")
                for k in range(KP)]
        wf8l = [p_wf.tile([128, 2, F], f8, name=f"wfl{k}", tag=f"wfl{k}")
                for k in range(KP)]

        # attention-phase inputs (freed before FFN)
        p_xh = tc.alloc_tile_pool(name="pxh", bufs=1, side="left")
        xh8 = [p_xh.tile([128, 2, T], f8, name=f"xh{k}", tag=f"xh{k}")
               for k in range(KP)]
        p_xq = tc.alloc_tile_pool(name="pxq", bufs=1, side="left")
        xq8 = [p_xq.tile([128, 2, Q], f8, name=f"xq{k}", tag=f"xq{k}")
               for k in range(KP)]
        p_wq = tc.alloc_tile_pool(name="pwq", bufs=1, side="left")
        wq8 = [p_wq.tile([128, 2, C], f8, name=f"wq{k}", tag=f"wq{k}")
               for k in range(KP)]
        p_wk = tc.alloc_tile_pool(name="pwk", bufs=1, side="left")
        wk8 = [p_wk.tile([128, 2, C], f8, name=f"wk{k}", tag=f"wk{k}")
               for k in range(KP)]
        p_wv = tc.alloc_tile_pool(name="pwv", bufs=1, side="left")
        wv8 = [p_wv.tile([128, 2, C], f8, name=f"wv{k}", tag=f"wv{k}")
               for k in range(KP)]

        for k in range(KP):
            r = slice(k * 128, (k + 1) * 128)
            nc.sync.dma_start(xq8[k][:], xq_d[r, :])
            nc.sync.dma_start(wq8[k][:], wq_d[r, :])
        for k in range(KP):
            r = slice(k * 128, (k + 1) * 128)
            nc.sync.dma_start(xh8[k][:], xh_d[r, :])
            nc.sync.dma_start(wv8[k][:], wv_d[r, :])
        for k in range(KP):
            r = slice(k * 128, (k + 1) * 128)
            nc.sync.dma_start(wk8[k][:], wk_d[r, :])

        # persistent activation tiles
        p_qt = tc.alloc_tile_pool(name="pqt", bufs=1, side="left")
        qT = [p_qt.tile([128, Q], bf16, name=f"qT{m}", tag=f"qT{m}")
              for m in range(8)]
        p_kt = tc.alloc_tile_pool(name="pkt", bufs=1, side="left")
        kT = [p_kt.tile([128, 4, 512], bf16, name=f"kT{m}", tag=f"kT{m}")
              for m in range(8)]
        p_v = tc.alloc_tile_pool(name="pv8", bufs=1, side="left")
        v8 = [p_v.tile([128, 2, H, DH + 1], f8, name=f"v8{p}", tag=f"v8{p}")
              for p in range(8)]
        p_yt = tc.alloc_tile_pool(name="pyt", bufs=1, side="right")
        yt8 = [p_yt.tile([128, 2, Q], f8, name=f"yt{p}", tag=f"yt{p}")
               for p in range(KP)]
        p_wp = tc.alloc_tile_pool(name="pwp", bufs=1, side="right")
        wp8 = [p_wp.tile([128, 2, C], f8, name=f"wp{k}", tag=f"wp{k}")
               for k in range(KP)]
        for k in range(KP):
            nc.sync.dma_start(wp8[k][:], wp_d[k * 128:(k + 1) * 128, :])
        # FFN weight prefetch: emitted after all projection inputs so the
        # (serialized) DMA engines load those first; overlaps attention.
        for k in range(KP):
            r = slice(k * 128, (k + 1) * 128)
            nc.sync.dma_start(wf8h[k][:], wfh_d[r, :])
        for k in range(KP):
            r = slice(k * 128, (k + 1) * 128)
            nc.sync.dma_start(wf8l[k][:], wfl_d[r, :])

        # ---------------- phase 1: Q and V projections ----------------
        with tc.tile_pool(name="ppq", bufs=2, space="PSUM") as ppq:
            for ft in range(8):
                fs = slice(ft * 128, (ft + 1) * 128)
                ps = ppq.tile([128, Q], f32, name="pq", tag="pq")
                for k in range(KP):
                    nc.tensor.matmul(ps[:], wq8[k][:, :, fs], xq8[k][:],
                                     start=(k == 0), stop=(k == KP - 1),
                                     perf_mode=DR)
                nc.vector.tensor_scalar_mul(qT[ft][:], ps[:], 1.0 / SQ)

        # V: out[keys, vfeat] via xhat-stationary; vfeat = 64h+d, +65th ones col
        with tc.tile_pool(name="ppv", bufs=2, space="PSUM") as ppv:
            for kt in range(16):
                ks = slice(kt * 128, (kt + 1) * 128)
                ps = ppv.tile([128, 16, 64], f32, name="pvv", tag="pvv")
                for vc in range(2):
                    for k in range(KP):
                        nc.tensor.matmul(ps[:, vc * 8:(vc + 1) * 8, :],
                                         xh8[k][:, :, ks],
                                         wv8[k][:, :, vc * 512:(vc + 1) * 512],
                                         start=(k == 0), stop=(k == KP - 1),
                                         perf_mode=DR)
                # [128, 16, 64] -> v8[kt//2][:, kt%2, :, 0:64] (stride 65)
                nc.vector.tensor_scalar_mul(
                    v8[kt // 2][:, kt % 2, :, 0:DH], ps[:], SY / SV)
            for p in range(8):
                nc.vector.memset(v8[p][:, :, :, DH:DH + 1], 1.0)

        # ---------------- phase 2: attention (K proj interleaved) ----------
        # per head h: level t: scores into psum group(s), exp -> a8, AV-DR.
        # K projection for feature tile ft is emitted just before the scores
        # of its two heads, feeding the PE while Act drains exp work.
        nheads = H

        with tc.tile_pool(name="psc", bufs=2, space="PSUM") as psc, \
             tc.tile_pool(name="psy", bufs=3, space="PSUM") as psy, \
             tc.tile_pool(name="pa8", bufs=3, side="right") as pa8, \
             tc.tile_pool(name="pnrm", bufs=2, side="right") as pnrm, \
             tc.tile_pool(name="ppk", bufs=1, space="PSUM") as ppk:

            def kproj(ft):
                fs = slice(ft * 128, (ft + 1) * 128)
                for kc in range(4):
                    ps = ppk.tile([128, 512], f32, name="pk", tag="pk")
                    cs = slice(kc * 512, (kc + 1) * 512)
                    for k in range(KP):
                        nc.tensor.matmul(ps[:], wk8[k][:, :, fs],
                                         xh8[k][:, :, cs],
                                         start=(k == 0),
                                         stop=(k == KP - 1),
                                         perf_mode=DR)
                    nc.vector.tensor_scalar_mul(
                        kT[ft][:, kc, :], ps[:], 1.0 / SK)

            def scores_head(h, y_ps):
                ft, po = h // 2, (h % 2) * 64
                groups = []
                for t in range(4):
                    nslot = 4 * (t + 1)
                    qs = slice(t * 128, (t + 1) * 128)
                    for g0 in range(0, nslot, 8):
                        gn = min(8, nslot - g0)
                        s_ps = psc.tile([128, 8, 128], f32, name="s", tag="s")
                        a8 = pa8.tile([128, 8, 128], f8, name="a8", tag="a8")
                        for si in range(gn):
                            s = g0 + si
                            masked = s >= nslot - 4
                            if masked:
                                nc.tensor.matmul(
                                    s_ps[:, si, :], eye[:],
                                    msk[:, s - (nslot - 4), :],
                                    start=True, stop=False)
                            nc.tensor.matmul(
                                s_ps[:, si, :],
                                kT[ft][po:po + 64, s // 4,
                                       (s % 4) * 128:(s % 4) * 128 + 128],
                                qT[ft][po:po + 64, qs],
                                start=not masked, stop=True)
                        nc.scalar.activation(a8[:, 0:gn, :], s_ps[:, 0:gn, :],
                                             Act.Exp, bias=shift_t[:])
                        groups.append((t, g0, gn, a8))
                return groups

            def av_head(h, y_ps, groups):
                for (t, g0, gn, a8) in groups:
                    qs = slice(t * 128, (t + 1) * 128)
                    npair_t = 2 * (t + 1)
                    for pi in range(gn // 2):
                        p = (g0 // 2) + pi
                        nc.tensor.matmul(
                            y_ps[0:65, qs], v8[p][:, :, h, :],
                            a8[:, 2 * pi:2 * pi + 2, :],
                            start=(p == 0), stop=(p == npair_t - 1),
                            perf_mode=DR)

            def norm_head(h, y_ps):
                rd = pnrm.tile([1, Q], f32, name="rd", tag="rd")
                rdb = pnrm.tile([1, Q], bf16, name="rdb", tag="rdb")
                rb = pnrm.tile([64, Q], bf16, name="rb", tag="rb")
                nc.vector.reciprocal(rd[:], y_ps[64:65, :])
                nc.vector.tensor_copy(rdb[:], rd[:])
                nc.tensor.matmul(y_ps[64:128, :], ones_r64[:], rdb[:],
                                 start=True, stop=True)
                nc.scalar.copy(rb[:], y_ps[64:128, :])
                pp, ss, rr = h // 4, (h // 2) % 2, (h % 2) * 64
                nc.vector.tensor_tensor(
                    yt8[pp][rr:rr + 64, ss, :], y_ps[0:64, :], rb[:],
                    Alu.mult)

            prev = None
            for h in range(nheads):
                if h % 2 == 0:
                    kproj(h // 2)
                y_ps = psy.tile([128, Q], f32, name="y", tag="y")
                groups = scores_head(h, y_ps)
                if prev is not None:
                    av_head(prev[0], prev[1], prev[2])
                    norm_head(prev[0], prev[1])
                prev = (h, y_ps, groups)
            av_head(prev[0], prev[1], prev[2])
            norm_head(prev[0], prev[1])

        p_v.release()
        p_kt.release()
        p_qt.release()
        p_wv.release()
        p_wk.release()
        p_wq.release()
        p_xq.release()
        p_xh.release()

        # ---------------- phase 3: attn proj + residual + LN2 ----------------
        p_x2 = tc.alloc_tile_pool(name="px2", bufs=1, side="right")
        xT_sb = [p_x2.tile([128, Q], f32, name=f"xT{m}", tag=f"xT{m}")
                 for m in range(8)]
        for m in range(8):
            nc.sync.dma_start(xT_sb[m][:], xT_d[m * 128:(m + 1) * 128, :])
        x2_sb = [p_x2.tile([128, Q], f32, name=f"x2{m}", tag=f"x2{m}")
                 for m in range(8)]
        x2b = [p_x2.tile([128, Q], bf16, name=f"x2b{m}", tag=f"x2b{m}")
               for m in range(8)]
        xq2h = [p_x2.tile([128, 2, Q], f8, name=f"q2h{k}", tag=f"q2h{k}")
                for k in range(KP)]
        xq2l = [p_x2.tile([128, 2, Q], f8, name=f"q2l{k}", tag=f"q2l{k}")
                for k in range(KP)]
        mu2 = p_x2.tile([1, Q], f32, name="mu2", tag="mu2")
        e22 = p_x2.tile([1, Q], f32, name="e22", tag="e22")
        rr2 = p_x2.tile([1, Q], f32, name="rr2", tag="rr2")
        mr2 = p_x2.tile([1, Q], f32, name="mr2", tag="mr2")
        mr2b = p_x2.tile([1, Q], bf16, name="mr2b", tag="mr2b")
        rr2b = p_x2.tile([1, Q], bf16, name="rr2b", tag="rr2b")
        S_sb = p_x2.tile([128, Q], bf16, name="S", tag="S")      # w2 x r2
        M_sb = p_x2.tile([128, Q], bf16, name="M", tag="M")      # w2 x mu2 r2 - b2
        t2_sb = p_x2.tile([128, Q], bf16, name="t2", tag="t2")

        with tc.tile_pool(name="pp3", bufs=2, space="PSUM") as pp3, \
             tc.tile_pool(name="pst", bufs=1, space="PSUM") as pst, \
             tc.tile_pool(name="psq", bufs=2, side="right") as psq:
            s2_ps = pst.tile([1, Q], f32, name="s2", tag="s2")
            q2_ps = pst.tile([1, Q], f32, name="q2", tag="q2")
            for ft in range(8):
                fs = slice(ft * 128, (ft + 1) * 128)
                ps = pp3.tile([128, Q], f32, name="pj", tag="pj")
                for k in range(KP):
                    nc.tensor.matmul(ps[:], wp8[k][:, :, fs], yt8[k][:],
                                     start=(k == 0), stop=(k == KP - 1),
                                     perf_mode=DR)
                nc.vector.scalar_tensor_tensor(
                    x2_sb[ft][:], ps[:], 1.0 / (SY * SY * SP), xT_sb[ft][:],
                    Alu.mult, Alu.add)
                nc.vector.tensor_copy(x2b[ft][:], x2_sb[ft][:])
                sq = psq.tile([128, Q], bf16, name="sq", tag="sq")
                nc.scalar.square(sq[:], x2b[ft][:])
                nc.tensor.matmul(s2_ps[:], ones_col[:], x2b[ft][:],
                                 start=(ft == 0), stop=(ft == 7))
                nc.tensor.matmul(q2_ps[:], ones_col[:], sq[:],
                                 start=(ft == 0), stop=(ft == 7))
            nc.vector.tensor_scalar_mul(mu2[:], s2_ps[:], 1.0 / C)
            nc.vector.tensor_scalar_mul(e22[:], q2_ps[:], 1.0 / C)

        # rr2 = 1/sqrt(var+eps); mr2 = mu2*rr2
        eps_t = p_x2.tile([1, 1], f32, name="eps", tag="eps")
        nc.vector.memset(eps_t[:], LN_EPS)
        nc.vector.tensor_tensor(rr2[:], mu2[:], mu2[:], Alu.mult)
        nc.vector.tensor_tensor(rr2[:], e22[:], rr2[:], Alu.subtract)
        nc.scalar.activation(rr2[:], rr2[:], Act.Sqrt, bias=eps_t[:])
        nc.vector.reciprocal(rr2[:], rr2[:])
        nc.vector.tensor_tensor(mr2[:], mu2[:], rr2[:], Alu.mult)
        nc.vector.tensor_copy(rr2b[:], rr2[:])
        nc.vector.tensor_copy(mr2b[:], mr2[:])

        # S = w2 (x) rr2 ; M = w2 (x) mr2 - b2 (x) 1   (PE broadcasts)
        w2b = p_x2.tile([1, C], bf16, name="w2b", tag="w2b")
        b2b = p_x2.tile([1, C], bf16, name="b2b", tag="b2b")
        onerow = p_x2.tile([1, Q], bf16, name="onerow", tag="onerow")
        nc.vector.tensor_copy(w2b[:], w2r[:])
        nc.vector.tensor_scalar_mul(b2b[:], b2r[:], -1.0)
        nc.vector.memset(onerow[:], 1.0)
        with tc.tile_pool(name="pbc2", bufs=2, space="PSUM") as pbc2:
            for ft in range(8):
                fs = slice(ft * 128, (ft + 1) * 128)
                sp = pbc2.tile([128, Q], f32, name="sp", tag="sp")
                mp = pbc2.tile([128, Q], f32, name="mp", tag="mp")
                nc.tensor.matmul(sp[:], w2b[:, fs], rr2b[:],
                                 start=True, stop=True)
                nc.tensor.matmul(mp[:], w2b[:, fs], mr2b[:],
                                 start=True, stop=False)
                nc.tensor.matmul(mp[:], b2b[:, fs], onerow[:],
                                 start=False, stop=True,
                                 skip_group_check=True)
                nc.scalar.copy(S_sb[:], sp[:])
                nc.scalar.copy(M_sb[:], mp[:])
                # xhat2 = x2*S - M, then hi/lo fp8 split
                kp, sl = ft // 2, ft % 2
                nc.vector.tensor_tensor(t2_sb[:], x2_sb[ft][:], S_sb[:],
                                        Alu.mult)
                nc.vector.tensor_tensor(t2_sb[:], t2_sb[:], M_sb[:],
                                        Alu.subtract)
                nc.vector.tensor_scalar_mul(xq2h[kp][:, sl, :], t2_sb[:], SA)
                nc.vector.scalar_tensor_tensor(
                    xq2l[kp][:, sl, :], t2_sb[:], SA,
                    xq2h[kp][:, sl, :], Alu.mult, Alu.subtract)

        # ---------------- phase 4: FFN ----------------
        p_g = tc.alloc_tile_pool(name="pg", bufs=1, side="right")
        g8h = [p_g.tile([128, 2, Q], f8, name=f"g8h{k}", tag=f"g8h{k}")
               for k in range(FKP)]
        g8l = [p_g.tile([128, 2, Q], f8, name=f"g8l{k}", tag=f"g8l{k}")
               for k in range(FKP)]
        with tc.tile_pool(name="pgb", bufs=4, side="right") as pgb, \
             tc.tile_pool(name="pph", bufs=3, space="PSUM") as pph:
            gb = None
            for ht in range(32):
                hs = slice(ht * 128, (ht + 1) * 128)
                ps = pph.tile([128, Q], f32, name="ph", tag="ph")
                for k in range(KP):
                    nc.tensor.matmul(ps[:], wf8h[k][:, :, hs], xq2h[k][:],
                                     start=(k == 0), stop=False,
                                     perf_mode=DR)
                for k in range(KP):
                    nc.tensor.matmul(ps[:], wf8h[k][:, :, hs], xq2l[k][:],
                                     start=False, stop=False, perf_mode=DR)
                for k in range(KP):
                    nc.tensor.matmul(ps[:], wf8l[k][:, :, hs], xq2h[k][:],
                                     start=False, stop=(k == KP - 1),
                                     perf_mode=DR)
                if ht % 2 == 0:
                    gb = pgb.tile([128, 2, Q], bf16, name="gb", tag="gb")
                nc.scalar.activation(gb[:, ht % 2, :], ps[:],
                                     Act.Gelu_apprx_tanh,
                                     scale=1.0 / (SA * SF))
                if ht % 2 == 1:
                    kp = ht // 2
                    nc.vector.tensor_scalar_mul(g8h[kp][:], gb[:], SA)
                    nc.vector.scalar_tensor_tensor(
                        g8l[kp][:], gb[:], SA, g8h[kp][:],
                        Alu.mult, Alu.subtract)
        p_wf.release()

        with tc.tile_pool(name="pwo", bufs=8, side="left") as p_wo, \
             tc.tile_pool(name="ppo", bufs=1, space="PSUM") as ppo, \
             tc.tile_pool(name="pout", bufs=4, side="right") as pout:
            o_ps = [ppo.tile([128, Q], f32, name=f"o{m}", tag=f"o{m}")
                    for m in range(8)]
            for k in range(FKP):
                r = slice(k * 128, (k + 1) * 128)
                woh_t = p_wo.tile([128, 2, C], f8, name="woh", tag="woh")
                wol_t = p_wo.tile([128, 2, C], f8, name="wol", tag="wol")
                nc.sync.dma_start(woh_t[:], woh_d[r, :])
                nc.sync.dma_start(wol_t[:], wol_d[r, :])
                for ft in range(8):
                    fs = slice(ft * 128, (ft + 1) * 128)
                    nc.tensor.matmul(o_ps[ft][:], woh_t[:, :, fs], g8h[k][:],
                                     start=(k == 0), stop=False,
                                     perf_mode=DR)
                    nc.tensor.matmul(o_ps[ft][:], woh_t[:, :, fs], g8l[k][:],
                                     start=False, stop=False, perf_mode=DR)
                    nc.tensor.matmul(o_ps[ft][:], wol_t[:, :, fs], g8h[k][:],
                                     start=False, stop=(k == FKP - 1),
                                     perf_mode=DR)
                    if k == FKP - 1:
                        # evacuate + store as soon as this column finishes
                        ot = pout.tile([128, Q], f32, name="ot", tag="ot")
                        nc.vector.scalar_tensor_tensor(
                            ot[:], o_ps[ft][:], 1.0 / (SA * SO),
                            x2_sb[ft][:], Alu.mult, Alu.add)
                        nc.sync.dma_start(
                            out_d[ft * 128:(ft + 1) * 128, :], ot[:])

        p_g.release()
        p_x2.release()
        p_wp.release()
        p_yt.release()
        cst.release()

    nc.compile()
    return nc


def _prep_inputs(x, w_attn, w_proj, w_fc, w_fc_proj,
                 ln1_w, ln1_b, ln2_w, ln2_b):
    f8 = ml_dtypes.float8_e4m3
    scale = 1.0 / math.sqrt(DH)

    def pack_w(W, npair):
        # [K, N] -> [npair*128, 2*N] pair-interleaved rows
        K, N = W.shape
        return np.ascontiguousarray(
            W.reshape(npair, 2, 128, N).transpose(0, 2, 1, 3)
             .reshape(npair * 128, 2 * N))

    def hi_lo(W):
        hi = W.astype(f8)
        lo = (W - hi.astype(np.float32)).astype(f8)
        return hi, lo

    wq = pack_w(w_attn[:, :C] * (scale * SQ), KP).astype(f8)
    wk = pack_w(w_attn[:, C:2 * C] * SK, KP).astype(f8)
    wv = pack_w(w_attn[:, 2 * C:] * SV, KP).astype(f8)
    wp = pack_w(w_proj * SP, KP).astype(f8)
    wfh, wfl = hi_lo(pack_w(w_fc * SF, KP))
    woh, wol = hi_lo(pack_w(w_fc_proj * SO, FKP))

    eye = np.eye(128, dtype=ml_dtypes.bfloat16)
    tri = np.where(np.arange(128)[:, None] > np.arange(128)[None, :],
                   np.float32(NEG), np.float32(0.0))  # [k, q]: NEG if q < k

    # host LN1
    mu = x.mean(axis=2, keepdims=True)
    var = ((x - mu) ** 2).mean(axis=2, keepdims=True)
    xhat = ((x - mu) / np.sqrt(var + LN_EPS)) * ln1_w + ln1_b  # [B, T, C]

    in_maps = []
    for c in range(NCORES):
        b, j = c // 4, c % 4
        qsel = np.concatenate([np.arange(128 * (4 * t + j),
                                         128 * (4 * t + j) + 128)
                               for t in range(4)])
        xh = pack_w(np.ascontiguousarray(xhat[b].T), KP).astype(f8)
        xq = pack_w(np.ascontiguousarray(xhat[b][qsel].T), KP).astype(f8)
        xT = np.ascontiguousarray(x[b][qsel].T.astype(np.float32))
        m = np.zeros((128, 4, 128), dtype=np.float32)
        for i in range(4):
            if i == j:
                m[:, i, :] = tri
            elif i > j:
                m[:, i, :] = NEG
        in_maps.append({
            "xh": xh, "xq": xq, "wq": wq, "wk": wk, "wv": wv, "wp": wp,
            "wfh": wfh, "wfl": wfl, "woh": woh, "wol": wol,
            "msk": m.reshape(128, 512).astype(ml_dtypes.bfloat16),
            "eye": eye,
            "w2r": ln2_w.reshape(1, C).astype(np.float32),
            "b2r": ln2_b.reshape(1, C).astype(np.float32),
            "xT": xT,
        })
    return in_maps


def _get_nc():
    if "nc" not in _cache:
        _cache["nc"] = _build()
    return _cache["nc"]


def _get_runner():
    if "runner" in _cache:
        return _cache["runner"]
    import jax
    import numpy as _np
    from jax.sharding import Mesh, PartitionSpec
    try:
        from jax.experimental.shard_map import shard_map
    except ImportError:
        from jax.shard_map import shard_map
    import concourse.mybir as mybir
    from concourse import bass2jax

    nc = _get_nc()
    bass2jax.install_neuronx_cc_hook()

    partition_name = (nc.partition_id_tensor.name
                      if nc.partition_id_tensor else None)
    in_names, out_names, out_avals, zero_outs = [], [], [], []
    for alloc in nc.m.functions[0].allocations:
        if not isinstance(alloc, mybir.MemoryLocationSet):
            continue
        name = alloc.memorylocations[0].name
        if alloc.kind == "ExternalInput":
            if name != partition_name:
                in_names.append(name)
        elif alloc.kind == "ExternalOutput":
            shape = tuple(alloc.tensor_shape)
            dtype = mybir.dt.np(alloc.dtype)
            out_names.append(name)
            out_avals.append(jax.core.ShapedArray(shape, dtype))
            zero_outs.append(_np.zeros(shape, dtype))
    n_params = len(in_names)
    n_outs = len(out_avals)
    all_in_names = list(in_names) + list(out_names)
    if partition_name is not None:
        all_in_names.append(partition_name)
    donate = tuple(range(n_params, n_params + n_outs))

    def _body(*args):
        operands = list(args)
        if partition_name is not None:
            operands.append(bass2jax.partition_id_tensor())
        outs = bass2jax._bass_exec_p.bind(
            *operands,
            out_avals=tuple(out_avals),
            in_names=tuple(all_in_names),
            out_names=tuple(out_names),
            lowering_input_output_aliases=(),
            sim_require_finite=True,
            sim_require_nnan=True,
            nc=nc,
        )
        return tuple(outs)

    devices = jax.devices()[:NCORES]
    mesh = Mesh(_np.asarray(devices), ("core",))
    in_specs = (PartitionSpec("core"),) * (n_params + n_outs)
    out_specs = (PartitionSpec("core"),) * n_outs
    sharded = jax.jit(
        shard_map(_body, mesh=mesh, in_specs=in_specs, out_specs=out_specs,
                  check_rep=False),
        donate_argnums=donate, keep_unused=True)

    def run(in_maps):
        concat_in = [
            _np.concatenate([_np.asarray(in_maps[c][n])
                             for c in range(NCORES)], axis=0)
            for n in in_names
        ]
        concat_zeros = [
            _np.zeros((NCORES * z.shape[0], *z.shape[1:]), z.dtype)
            for z in zero_outs
        ]
        out_arrs = sharded(*concat_in, *concat_zeros)
        return [
            {n: _np.asarray(out_arrs[i]).reshape(
                NCORES, *out_avals[i].shape)[c]
             for i, n in enumerate(out_names)}
            for c in range(NCORES)
        ]

    _cache["runner"] = run
    return run


def kernel(x, w_attn, w_proj, w_fc, w_fc_proj, ln1_w, ln1_b, ln2_w, ln2_b):
    x = np.asarray(x, dtype=np.float32)
    in_maps = _prep_inputs(
        x, np.asarray(w_attn, np.float32), np.asarray(w_proj, np.float32),
        np.asarray(w_fc, np.float32), np.asarray(w_fc_proj, np.float32),
        np.asarray(ln1_w, np.float32), np.asarray(ln1_b, np.float32),
        np.asarray(ln2_w, np.float32), np.asarray(ln2_b, np.float32))
    results = _get_runner()(in_maps)
    out = np.empty((B, T, C), dtype=np.float32)
    for c in range(NCORES):
        b, j = c // 4, c % 4
        o = results[c]["outT"]  # [C, Q]
        for t in range(4):
            qt = 4 * t + j
            out[b, 128 * qt:128 * qt + 128, :] = o[:, 128 * t:128 * t + 128].T
    return out
